# revision 7
# baseline (speedup 1.0000x reference)
"""AMSoftmax (norm-free branch) Trainium2 kernel, 8 NeuronCores.

Reference computes, for input x [B,D], label [B], weight [C,D], scalars s,m:
    norm   = ||x||_2 per row                       [B,1]
    cosine = (x/max(norm,eps)) @ (w/max(||w||,eps)).T   [B,C]
    logits = norm * (cosine - m*onehot(label))     [B,C]
    returns (logits, cosine)

Key identity: norm * cosine == x @ w_hat.T exactly, so per output element:
    raw    = x @ w_hat.T          (PSUM, f32)
    cosine = raw * (1/norm)       (per-row scale, ACT)
    logits = raw - norm*m*onehot  (DVE sub against a sparse mask)

Sharding: 2-way over batch x 4-way over classes (8 cores, no collectives;
outputs are disjoint tiles concatenated on host). Per core: x [2048,512],
w_hat [2000,512], outputs [2048,2000] each, stored as bf16.

Schedule (v3): quarter-width (500-col) single-bank PSUM groups with a
6-deep psM pool so PE decouples from ACT/DVE consumer jitter; x casts on
GPSIMD; W tiles 0-3 as single-tile DMA descriptors (per-descriptor drain
is ~70GB/s, parallelism comes from ~5 in flight); stores on the sync ring
strictly after all input dispatches; PE warm-up burst on the identity.
"""

import os
import sys

sys.path.insert(0, "/opt/trn_rl_repo")

import numpy as np

B, D, C = 4096, 512, 8000
NB, NCL = 2, 4  # batch x class core grid
BL, CL = B // NB, C // NCL  # 2048, 2000 per core
RT = BL // 128  # 16 row tiles
KC = D // 128  # 4 contraction chunks
CW = 500  # matmul free-dim chunk (PSUM bank holds 512 f32)
HW_ = 2 * CW  # 1000 columns per half
NH = CL // HW_  # 2 column halves per row tile

COMPUTE = os.environ.get("AMS_DTYPE", "bf16")
OUT_BF16 = os.environ.get("AMS_OUT", "bf16") == "bf16"
WARMUP_MM = int(os.environ.get("AMS_WARMUP", "24"))
CATCH = int(os.environ.get("AMS_CATCH", "4"))  # h=1 catch-up offset

_CACHE = {}


def _build():
    import concourse.mybir as mybir
    import concourse.tile as tile
    from concourse import bacc, library_config
    from concourse.masks import make_identity

    f32 = mybir.dt.float32
    i16 = mybir.dt.int16
    bf16 = mybir.dt.bfloat16
    cdt = bf16 if COMPUTE == "bf16" else mybir.dt.float32r
    odt = bf16 if OUT_BF16 else f32

    nc = bacc.Bacc()
    x_ext = nc.declare_dram_parameter("x", [BL, D], f32, isOutput=False)
    w_ext = nc.declare_dram_parameter("w", [CL, D], f32, isOutput=False)
    labx_ext = nc.declare_dram_parameter("labx", [128, 2 * RT], i16, isOutput=False)
    m_ext = nc.declare_dram_parameter("mvec", [128, 1], f32, isOutput=False)
    logits_ext = nc.declare_dram_parameter("logits", [BL, CL], odt, isOutput=True)
    cosine_ext = nc.declare_dram_parameter("cosine", [BL, CL], odt, isOutput=True)

    WT = (CL + 127) // 128  # 16 w row tiles (last one 80 partitions)

    with tile.TileContext(nc) as tc:
        with (
            tc.tile_pool(name="persist", bufs=1) as persist,
            tc.tile_pool(name="sq", bufs=4) as sq_pool,
            tc.tile_pool(name="xbf", bufs=3) as xbf_pool,
            tc.tile_pool(name="psT", bufs=2, space="PSUM") as psT_pool,
            tc.tile_pool(name="psM", bufs=6, space="PSUM") as psM_pool,
            tc.tile_pool(name="outb", bufs=6) as out_pool,
            tc.tile_pool(name="mm", bufs=7) as mm_pool,
        ):
            # tiny labx/m loads ride the gpsimd SWDGE before the scatter
            # library replaces the mainline ucode
            labx_sb = persist.tile([128, 2 * RT], i16)
            m_sb = persist.tile([128, 1], f32)
            nc.gpsimd.dma_start(labx_sb[:], labx_ext[:])
            nc.gpsimd.dma_start(m_sb[:], m_ext[:])

            nc.gpsimd.load_library(library_config.local_scatter)
            identity = persist.tile([128, 128], cdt)
            make_identity(nc, identity)

            w_in = persist.tile([128, WT, D], f32)
            x_in = persist.tile([128, RT, D], f32)
            w_bf = persist.tile([128, WT, D], cdt)
            wts = []
            for k in range(KC):
                wts.append(persist.tile([128, CL], cdt, tag=f"wt{k}", name=f"wt{k}"))
            xts = []
            for t in range(RT):
                xts.append(persist.tile([128, KC, 128], cdt, tag=f"xt{t}", name=f"xt{t}"))

            xss = persist.tile([128, RT], f32)
            xnorm = persist.tile([128, RT], f32)
            inv_xnorm = persist.tile([128, RT], f32)
            norm_m = persist.tile([128, RT], f32)
            normm2 = persist.tile([128, 2 * RT], bf16)
            wss = persist.tile([128, WT], f32)
            inv_wnorm = persist.tile([128, WT], f32)

            nc.vector.memset(w_in[64:, WT - 1, :], 0.0)
            nc.vector.memset(wss[:], 1.0)

            # ---- x DMAs on the scalar HWDGE ring: row tiles 0 and 1 as
            # separate descriptors up front, then ACT table preloads in the
            # transfer shadow, then the rest paced ----
            def x_load(t0, t1):
                nc.scalar.dma_start(
                    x_in[:, t0:t1, :],
                    x_ext[128 * t0 : 128 * t1, :].rearrange(
                        "(a p) d -> p a d", p=128
                    ),
                )

            x_load(0, 1)
            x_load(1, 2)
            dumm = persist.tile([128, 1], f32)
            nc.scalar.sqrt(dumm[:], wss[:, :1])
            nc.scalar.copy(dumm[:], wss[:, :1])
            x_load(2, 4)

            # ---- W DMAs on the sync HWDGE ring: tiles 0-3 as single-tile
            # descriptors (lower latency to first transposes), remaining
            # pairs as 0.5MB descriptors. Stores are emitted later on this
            # same ring so inputs always drain first. ----
            def w_load_tile(a):
                pa = min(128, CL - a * 128)
                nc.sync.dma_start(w_in[:pa, a, :], w_ext[128 * a : 128 * a + pa, :])

            def w_load_pair(pr):
                if pr < 7:
                    nc.sync.dma_start(
                        w_in[:, 2 * pr : 2 * pr + 2, :],
                        w_ext[256 * pr : 256 * (pr + 1), :].rearrange(
                            "(a p) d -> p a d", p=128
                        ),
                    )
                else:
                    nc.sync.dma_start(w_in[:, 14, :], w_ext[1792:1920, :])
                    nc.sync.dma_start(w_in[:80, 15, :], w_ext[1920:2000, :])

            for a in range(4):
                w_load_tile(a)
            for pr in range(2, 8):
                w_load_pair(pr)

            # ---- PE warm-up on the identity while DMAs land ----
            def warmup():
                ps = psM_pool.tile([128, 512], f32, tag="psM")
                for _ in range(WARMUP_MM):
                    nc.tensor.matmul(
                        ps[:, :128], identity[:], identity[:], start=True, stop=True
                    )

            # ---- W prep ----
            def w_norm_cast(pr):
                a, b = 2 * pr, 2 * pr + 1
                pa = min(128, CL - a * 128)
                pb = min(128, CL - b * 128)
                sqa = sq_pool.tile([128, D], f32, tag="sq")
                sqb = sq_pool.tile([128, D], f32, tag="sq")
                nc.scalar.activation(
                    sqa[:],
                    w_in[:, a, :],
                    mybir.ActivationFunctionType.Square,
                    accum_out=wss[:, a : a + 1],
                )
                nc.vector.tensor_mul(sqb[:], w_in[:, b, :], w_in[:, b, :])
                nc.vector.reduce_sum(
                    wss[:, b : b + 1], sqb[:], axis=mybir.AxisListType.X
                )
                cs = slice(a, b + 1)
                nc.scalar.sqrt(wss[:, cs], wss[:, cs])
                nc.vector.tensor_scalar_max(wss[:, cs], wss[:, cs], 1e-12)
                nc.vector.reciprocal(inv_wnorm[:, cs], wss[:, cs])
                nc.scalar.mul(
                    w_bf[:pa, a, :], w_in[:pa, a, :], inv_wnorm[:pa, a : a + 1]
                )
                nc.vector.tensor_scalar_mul(
                    w_bf[:pb, b, :], w_in[:pb, b, :], inv_wnorm[:pb, b : b + 1]
                )

            def w_tr(pr):
                a, b = 2 * pr, 2 * pr + 1
                pa = min(128, CL - a * 128)
                pb = min(128, CL - b * 128)
                for k in range(KC):
                    ps = psT_pool.tile([128, 2, 128], cdt, tag="psT")
                    nc.tensor.transpose(
                        ps[:, 0, :pa],
                        w_bf[:pa, a, k * 128 : (k + 1) * 128],
                        identity[:pa, :pa],
                    )
                    nc.tensor.transpose(
                        ps[:, 1, :pb],
                        w_bf[:pb, b, k * 128 : (k + 1) * 128],
                        identity[:pb, :pb],
                    )
                    eng = nc.vector.tensor_copy if pr % 2 == 0 else nc.scalar.copy
                    if pr < 7:
                        eng(wts[k][:, 256 * pr : 256 * (pr + 1)], ps[:])
                    else:
                        eng(wts[k][:, 1792:1920], ps[:, 0, :])
                        eng(wts[k][:, 1920:2000], ps[:, 1, :80])

            # ---- X prep: cast on GPSIMD, transpose on PE, copy ACT/DVE ----
            def x_sq(t, force_act=False):
                sq = sq_pool.tile([128, D], f32, tag="sq")
                if force_act or t % 2 == 0:
                    nc.scalar.activation(
                        sq[:],
                        x_in[:, t, :],
                        mybir.ActivationFunctionType.Square,
                        accum_out=xss[:, t : t + 1],
                    )
                else:
                    nc.vector.tensor_mul(sq[:], x_in[:, t, :], x_in[:, t, :])
                    nc.vector.reduce_sum(
                        xss[:, t : t + 1], sq[:], axis=mybir.AxisListType.X
                    )

            def x_cast(t):
                xb = xbf_pool.tile([128, D], cdt, tag="xb")
                nc.gpsimd.tensor_copy(xb[:], x_in[:, t, :])
                return xb

            def x_tr(t, xb):
                ps = psT_pool.tile([128, KC, 128], cdt, tag="psT")
                for k in range(KC):
                    nc.tensor.transpose(
                        ps[:, k, :], xb[:, k * 128 : (k + 1) * 128], identity[:]
                    )
                if t % 2 == 0:
                    nc.scalar.copy(xts[t][:], ps[:])
                else:
                    nc.vector.tensor_copy(xts[t][:], ps[:])

            def x_norms(g):
                cs = slice(2 * g, 2 * g + 2)
                nc.scalar.sqrt(xnorm[:, cs], xss[:, cs])
                nc.vector.tensor_scalar_max(xnorm[:, cs], xnorm[:, cs], 1e-12)
                nc.vector.reciprocal(inv_xnorm[:, cs], xnorm[:, cs])
                nc.vector.tensor_mul(
                    norm_m[:, cs], xnorm[:, cs], m_sb.broadcast_to([128, 2])
                )

            def normm_pair(t):
                nc.scalar.copy(
                    normm2[:, 2 * t : 2 * t + 2],
                    norm_m[:, t : t + 1].broadcast_to([128, 2]),
                )

            masks = {}

            def mask(t):
                mmt = mm_pool.tile([128, CL], bf16, tag="mm")
                nc.gpsimd.local_scatter(
                    mmt[:],
                    normm2[:, 2 * t : 2 * t + 2],
                    labx_sb[:, 2 * t : 2 * t + 2],
                    channels=128,
                    num_elems=CL,
                    num_idxs=2,
                )
                masks[t] = mmt

            # ---- main quarters: 4 accumulating matmuls into one PSUM bank ----
            outs = {}

            def mq(t, h, cc):
                ps = psM_pool.tile([128, 512], f32, tag="psM")
                c0 = h * HW_ + cc * CW
                for k in range(KC):
                    nc.tensor.matmul(
                        ps[:, :CW],
                        xts[t][:, k, :],
                        wts[k][:, c0 : c0 + CW],
                        start=(k == 0),
                        stop=(k == KC - 1),
                    )
                return ps

            def consume(t, h, cc, ps):
                if (t, h) not in outs:
                    cos_h = out_pool.tile([128, HW_], odt, tag="cos")
                    log_h = out_pool.tile([128, HW_], odt, tag="log")
                    outs[(t, h)] = (cos_h, log_h)
                cos_h, log_h = outs[(t, h)]
                sl = slice(cc * CW, (cc + 1) * CW)
                c0 = h * HW_ + cc * CW
                nc.scalar.activation(
                    cos_h[:, sl],
                    ps[:, :CW],
                    mybir.ActivationFunctionType.Copy,
                    scale=inv_xnorm[:, t : t + 1],
                )
                nc.vector.tensor_sub(
                    log_h[:, sl], ps[:, :CW], masks[t][:, c0 : c0 + CW]
                )

            def store(t, h):
                cos_h, log_h = outs.pop((t, h))
                if h == NH - 1:
                    masks.pop(t)
                r0, r1 = t * 128, (t + 1) * 128
                c0, c1 = h * HW_, (h + 1) * HW_
                nc.sync.dma_start(cosine_ext[r0:r1, c0:c1], cos_h[:])
                nc.sync.dma_start(logits_ext[r0:r1, c0:c1], log_h[:])

            def main_h(t, h):
                ps0 = mq(t, h, 0)
                consume(t, h, 0, ps0)
                ps1 = mq(t, h, 1)
                consume(t, h, 1, ps1)
                store(t, h)

            # ---- emission ----
            if WARMUP_MM:
                warmup()
            w_norm_cast(0)
            xb0 = x_cast(0)
            xb1 = x_cast(1)
            w_norm_cast(1)
            x_tr(0, xb0)
            w_tr(0)
            x_sq(0, force_act=True)
            x_tr(1, xb1)
            w_tr(1)
            x_sq(1)
            x_norms(0)
            normm_pair(0)
            normm_pair(1)
            mask(0)
            mask(1)
            # first two row tiles, quarter-interleaved with pair 2/3 prep
            # and the tile 2/3 lookahead
            ps = mq(0, 0, 0)
            w_norm_cast(2)
            consume(0, 0, 0, ps)
            ps = mq(1, 0, 0)
            w_tr(2)
            consume(1, 0, 0, ps)
            w_norm_cast(3)
            ps = mq(0, 0, 1)
            xb2 = x_cast(2)
            consume(0, 0, 1, ps)
            store(0, 0)
            w_tr(3)
            ps = mq(1, 0, 1)
            x_tr(2, xb2)
            consume(1, 0, 1, ps)
            store(1, 0)
            xb3 = x_cast(3)
            x_sq(2)
            x_tr(3, xb3)
            x_sq(3)
            x_norms(1)
            normm_pair(2)
            normm_pair(3)
            mask(2)
            mask(3)

            # sweep: iteration t runs main(t,0) and main(t-CATCH,1);
            # row tile t+2 is prepped during iteration t
            for t in range(2, RT + CATCH):
                t0 = t
                t1 = t - CATCH
                if t0 < RT:
                    if t0 - 2 < 4:
                        w_norm_cast(4 + (t0 - 2))
                    nt = t0 + 2
                    xb = None
                    if nt < RT:
                        if t0 % 2 == 0 and nt + 2 < RT:
                            x_load(nt + 2, min(nt + 4, RT))
                        xb = x_cast(nt)
                    ps = mq(t0, 0, 0)
                    consume(t0, 0, 0, ps)
                    ps = mq(t0, 0, 1)
                    if xb is not None:
                        x_tr(nt, xb)
                    consume(t0, 0, 1, ps)
                    store(t0, 0)
                    if t0 - 2 < 4:
                        w_tr(4 + (t0 - 2))
                    if nt < RT:
                        x_sq(nt)
                        if nt % 2 == 1:
                            x_norms(nt // 2)
                            normm_pair(nt - 1)
                            normm_pair(nt)
                            mask(nt - 1)
                            mask(nt)
                if 0 <= t1 < RT:
                    ps = mq(t1, 1, 0)
                    consume(t1, 1, 0, ps)
                    ps = mq(t1, 1, 1)
                    consume(t1, 1, 1, ps)
                    store(t1, 1)

    nc.finalize()
    return nc


def _in_maps(x, w, lab, mval):
    maps = []
    lab = np.asarray(lab).astype(np.int64)
    for ci in range(8):
        bi, cj = ci // NCL, ci % NCL
        b0, c0 = bi * BL, cj * CL
        ll = (lab[b0 : b0 + BL] - c0).reshape(RT, 128).T  # [128, RT]
        valid = (ll >= 0) & (ll < CL)
        labx = np.full((128, 2 * RT), -2, dtype=np.int16)
        labx[:, 0::2] = np.where(valid, ll, -1).astype(np.int16)
        maps.append(
            {
                "x": x[b0 : b0 + BL],
                "w": w[c0 : c0 + CL],
                "labx": np.ascontiguousarray(labx),
                "mvec": np.full((128, 1), mval, dtype=np.float32),
            }
        )
    return maps


def kernel(input, label, weight, s, m):
    from concourse.bass_utils import run_bass_kernel_spmd

    if "nc" not in _CACHE:
        _CACHE["nc"] = _build()
    nc = _CACHE["nc"]

    x = np.ascontiguousarray(np.asarray(input, dtype=np.float32))
    w = np.ascontiguousarray(np.asarray(weight, dtype=np.float32))
    lab = np.asarray(label)
    mval = float(np.asarray(m))

    res = run_bass_kernel_spmd(nc, _in_maps(x, w, lab, mval), core_ids=list(range(8)))

    logits = np.empty((B, C), dtype=np.float32)
    cosine = np.empty((B, C), dtype=np.float32)
    for ci in range(8):
        bi, cj = ci // NCL, ci % NCL
        b0, c0 = bi * BL, cj * CL
        logits[b0 : b0 + BL, c0 : c0 + CL] = np.asarray(
            res.results[ci]["logits"], dtype=np.float32
        )
        cosine[b0 : b0 + BL, c0 : c0 + CL] = np.asarray(
            res.results[ci]["cosine"], dtype=np.float32
        )
    return logits, cosine


# revision 8
# speedup vs baseline: 1.1205x; 1.1205x over previous
"""AMSoftmax (norm-free branch) Trainium2 kernel, 8 NeuronCores.

Reference computes, for input x [B,D], label [B], weight [C,D], scalars s,m:
    norm   = ||x||_2 per row                       [B,1]
    cosine = (x/max(norm,eps)) @ (w/max(||w||,eps)).T   [B,C]
    logits = norm * (cosine - m*onehot(label))     [B,C]
    returns (logits, cosine)

Key identity: norm * cosine == x @ w_hat.T exactly, so per output element:
    raw    = x @ w_hat.T          (PSUM, f32)
    cosine = raw * (1/norm)       (per-row scale, ACT)
    logits = raw - norm*m*onehot  (DVE sub against a sparse mask)

Sharding: 2-way over batch x 4-way over classes (8 cores, no collectives;
outputs are disjoint tiles concatenated on host). Per core: x [2048,512],
w_hat [2000,512], outputs [2048,2000] each, stored as bf16.

Schedule (v4): the whole W prep (sumsq on ACT, scale-cast on DVE, PE
transposes with one merged copy per pair) runs in the DMA-bound prologue,
so sweep iterations carry a uniform, PE-subcritical consumer load and the
PE main stream never starves (HAM stays warm). Identity-matmul warm-up
fills PE gaps in the prologue. Quarter-width single-bank PSUM groups with
a 6-deep pool decouple PE from consumer jitter. Inputs and stores share
the sync HWDGE ring in strict order (inputs first); x rides the scalar
ring.
"""

import os
import sys

sys.path.insert(0, "/opt/trn_rl_repo")

import numpy as np

B, D, C = 4096, 512, 8000
NB, NCL = 2, 4  # batch x class core grid
BL, CL = B // NB, C // NCL  # 2048, 2000 per core
RT = BL // 128  # 16 row tiles
KC = D // 128  # 4 contraction chunks
CW = 500  # matmul free-dim chunk (PSUM bank holds 512 f32)
HW_ = 2 * CW  # 1000 columns per half
NH = CL // HW_  # 2 column halves per row tile

COMPUTE = os.environ.get("AMS_DTYPE", "bf16")
OUT_BF16 = os.environ.get("AMS_OUT", "bf16") == "bf16"
WARMUP_MM = int(os.environ.get("AMS_WARMUP", "20"))
WARMUP2_MM = int(os.environ.get("AMS_WARMUP2", "24"))
CATCH = int(os.environ.get("AMS_CATCH", "1"))  # h=1 catch-up offset

_CACHE = {}


def _build():
    import concourse.mybir as mybir
    import concourse.tile as tile
    from concourse import bacc, library_config
    from concourse.masks import make_identity

    f32 = mybir.dt.float32
    i16 = mybir.dt.int16
    bf16 = mybir.dt.bfloat16
    cdt = bf16 if COMPUTE == "bf16" else mybir.dt.float32r
    odt = bf16 if OUT_BF16 else f32

    nc = bacc.Bacc()
    x_ext = nc.declare_dram_parameter("x", [BL, D], f32, isOutput=False)
    w_ext = nc.declare_dram_parameter("w", [CL, D], f32, isOutput=False)
    labx_ext = nc.declare_dram_parameter("labx", [128, 2 * RT], i16, isOutput=False)
    m_ext = nc.declare_dram_parameter("mvec", [128, 1], f32, isOutput=False)
    logits_ext = nc.declare_dram_parameter("logits", [BL, CL], odt, isOutput=True)
    cosine_ext = nc.declare_dram_parameter("cosine", [BL, CL], odt, isOutput=True)

    WT = (CL + 127) // 128  # 16 w row tiles (last one 80 partitions)

    with tile.TileContext(nc) as tc:
        with (
            tc.tile_pool(name="persist", bufs=1) as persist,
            tc.tile_pool(name="sq", bufs=4) as sq_pool,
            tc.tile_pool(name="xbf", bufs=3) as xbf_pool,
            tc.tile_pool(name="psT", bufs=2, space="PSUM") as psT_pool,
            tc.tile_pool(name="psM", bufs=6, space="PSUM") as psM_pool,
            tc.tile_pool(name="outb", bufs=6) as out_pool,
            tc.tile_pool(name="mm", bufs=6) as mm_pool,
        ):
            # tiny labx/m loads ride the gpsimd SWDGE before the scatter
            # library replaces the mainline ucode
            labx_sb = persist.tile([128, 2 * RT], i16)
            m_sb = persist.tile([128, 1], f32)
            nc.gpsimd.dma_start(labx_sb[:], labx_ext[:])
            nc.gpsimd.dma_start(m_sb[:], m_ext[:])

            nc.gpsimd.load_library(library_config.local_scatter)
            identity = persist.tile([128, 128], cdt)
            make_identity(nc, identity)

            w_in = persist.tile([128, WT, D], f32)
            x_in = persist.tile([128, RT, D], f32)
            w_bf = persist.tile([128, WT, D], cdt)
            wts = persist.tile([128, KC, CL], cdt)  # transposed normalized W
            xts = []
            for t in range(RT):
                xts.append(
                    persist.tile([128, KC, 128], cdt, tag=f"xt{t}", name=f"xt{t}")
                )

            xss = persist.tile([128, RT], f32)
            xnorm = persist.tile([128, RT], f32)
            inv_xnorm = persist.tile([128, RT], f32)
            norm_m = persist.tile([128, RT], f32)
            normm2 = persist.tile([128, 2 * RT], bf16)
            wss = persist.tile([128, WT], f32)
            inv_wnorm = persist.tile([128, WT], f32)

            nc.vector.memset(w_in[64:, WT - 1, :], 0.0)
            nc.vector.memset(wss[:], 1.0)

            # ---- x DMAs on the scalar HWDGE ring ----
            def x_load(t0, t1):
                nc.scalar.dma_start(
                    x_in[:, t0:t1, :],
                    x_ext[128 * t0 : 128 * t1, :].rearrange(
                        "(a p) d -> p a d", p=128
                    ),
                )

            x_load(0, 1)
            x_load(1, 2)
            dumm = persist.tile([128, 1], f32)
            nc.scalar.sqrt(dumm[:], wss[:, :1])
            nc.scalar.copy(dumm[:], wss[:, :1])
            x_load(2, 4)

            # ---- W DMAs on the sync HWDGE ring: tiles 0-3 as single-tile
            # descriptors, remaining pairs as 0.5MB descriptors; stores are
            # emitted later on this same ring so inputs drain first ----
            def w_load_tile(a):
                pa = min(128, CL - a * 128)
                nc.sync.dma_start(w_in[:pa, a, :], w_ext[128 * a : 128 * a + pa, :])

            def w_load_pair(pr):
                if pr < 7:
                    nc.sync.dma_start(
                        w_in[:, 2 * pr : 2 * pr + 2, :],
                        w_ext[256 * pr : 256 * (pr + 1), :].rearrange(
                            "(a p) d -> p a d", p=128
                        ),
                    )
                else:
                    nc.sync.dma_start(w_in[:, 14, :], w_ext[1792:1920, :])
                    nc.sync.dma_start(w_in[:80, 15, :], w_ext[1920:2000, :])

            for a in range(4):
                w_load_tile(a)
            for pr in range(2, 8):
                w_load_pair(pr)

            def warmup(n):
                ps = psM_pool.tile([128, 512], f32, tag="psM")
                for _ in range(n):
                    nc.tensor.matmul(
                        ps[:, :128], identity[:], identity[:], start=True, stop=True
                    )

            # ---- W prep: sumsq on ACT, sqrt on ACT, recip + scale-cast on
            # DVE, PE transposes with one merged PSUM->SBUF copy per pair ----
            def w_sumsq(pr):
                a, b = 2 * pr, 2 * pr + 1
                for c in (a, b):
                    sqc = sq_pool.tile([128, D], f32, tag="sq")
                    nc.scalar.activation(
                        sqc[:],
                        w_in[:, c, :],
                        mybir.ActivationFunctionType.Square,
                        accum_out=wss[:, c : c + 1],
                    )
                cs = slice(a, b + 1)
                nc.scalar.sqrt(wss[:, cs], wss[:, cs])

            def w_scale(pr):
                a, b = 2 * pr, 2 * pr + 1
                pa = min(128, CL - a * 128)
                pb = min(128, CL - b * 128)
                cs = slice(a, b + 1)
                nc.vector.tensor_scalar_max(wss[:, cs], wss[:, cs], 1e-12)
                nc.vector.reciprocal(inv_wnorm[:, cs], wss[:, cs])
                nc.vector.tensor_scalar_mul(
                    w_bf[:pa, a, :], w_in[:pa, a, :], inv_wnorm[:pa, a : a + 1]
                )
                nc.vector.tensor_scalar_mul(
                    w_bf[:pb, b, :], w_in[:pb, b, :], inv_wnorm[:pb, b : b + 1]
                )

            def w_tr(pr):
                a, b = 2 * pr, 2 * pr + 1
                pa = min(128, CL - a * 128)
                pb = min(128, CL - b * 128)
                ps = psT_pool.tile([128, KC, 256], cdt, tag="psT")
                for k in range(KC):
                    nc.tensor.transpose(
                        ps[:, k, :pa],
                        w_bf[:pa, a, k * 128 : (k + 1) * 128],
                        identity[:pa, :pa],
                    )
                    nc.tensor.transpose(
                        ps[:, k, 128 : 128 + pb],
                        w_bf[:pb, b, k * 128 : (k + 1) * 128],
                        identity[:pb, :pb],
                    )
                eng = nc.vector.tensor_copy if pr % 2 == 0 else nc.scalar.copy
                if pr < 7:
                    eng(wts[:, :, 256 * pr : 256 * (pr + 1)], ps[:])
                else:
                    eng(wts[:, :, 1792:1920], ps[:, :, :128])
                    eng(wts[:, :, 1920:2000], ps[:, :, 128:208])

            # ---- X prep ----
            def x_sq(t):
                sq = sq_pool.tile([128, D], f32, tag="sq")
                nc.scalar.activation(
                    sq[:],
                    x_in[:, t, :],
                    mybir.ActivationFunctionType.Square,
                    accum_out=xss[:, t : t + 1],
                )

            def x_cast(t):
                xb = xbf_pool.tile([128, D], cdt, tag="xb")
                nc.vector.tensor_copy(xb[:], x_in[:, t, :])
                return xb

            def x_tr(t, xb):
                ps = psT_pool.tile([128, KC, 128], cdt, tag="psT")
                for k in range(KC):
                    nc.tensor.transpose(
                        ps[:, k, :], xb[:, k * 128 : (k + 1) * 128], identity[:]
                    )
                if t % 2 == 0:
                    nc.scalar.copy(xts[t][:], ps[:])
                else:
                    nc.vector.tensor_copy(xts[t][:], ps[:])

            def x_norms(g):
                cs = slice(2 * g, 2 * g + 2)
                nc.scalar.sqrt(xnorm[:, cs], xss[:, cs])
                nc.vector.tensor_scalar_max(xnorm[:, cs], xnorm[:, cs], 1e-12)
                nc.vector.reciprocal(inv_xnorm[:, cs], xnorm[:, cs])
                nc.vector.tensor_mul(
                    norm_m[:, cs], xnorm[:, cs], m_sb.broadcast_to([128, 2])
                )

            def normm_pair(t):
                nc.scalar.copy(
                    normm2[:, 2 * t : 2 * t + 2],
                    norm_m[:, t : t + 1].broadcast_to([128, 2]),
                )

            masks = {}

            def mask(t):
                mmt = mm_pool.tile([128, CL], bf16, tag="mm")
                nc.gpsimd.local_scatter(
                    mmt[:],
                    normm2[:, 2 * t : 2 * t + 2],
                    labx_sb[:, 2 * t : 2 * t + 2],
                    channels=128,
                    num_elems=CL,
                    num_idxs=2,
                )
                masks[t] = mmt

            # ---- mains: quarter-width single-bank PSUM groups ----
            outs = {}

            def mq(t, h, cc):
                ps = psM_pool.tile([128, 512], f32, tag="psM")
                c0 = h * HW_ + cc * CW
                for k in range(KC):
                    nc.tensor.matmul(
                        ps[:, :CW],
                        xts[t][:, k, :],
                        wts[:, k, c0 : c0 + CW],
                        start=(k == 0),
                        stop=(k == KC - 1),
                    )
                return ps

            def consume(t, h, cc, ps):
                if (t, h) not in outs:
                    cos_h = out_pool.tile([128, HW_], odt, tag="cos")
                    log_h = out_pool.tile([128, HW_], odt, tag="log")
                    outs[(t, h)] = (cos_h, log_h)
                cos_h, log_h = outs[(t, h)]
                sl = slice(cc * CW, (cc + 1) * CW)
                c0 = h * HW_ + cc * CW
                nc.scalar.activation(
                    cos_h[:, sl],
                    ps[:, :CW],
                    mybir.ActivationFunctionType.Copy,
                    scale=inv_xnorm[:, t : t + 1],
                )
                nc.vector.tensor_sub(
                    log_h[:, sl], ps[:, :CW], masks[t][:, c0 : c0 + CW]
                )

            def store(t, h):
                cos_h, log_h = outs.pop((t, h))
                if h == NH - 1:
                    masks.pop(t)
                r0, r1 = t * 128, (t + 1) * 128
                c0, c1 = h * HW_, (h + 1) * HW_
                nc.sync.dma_start(cosine_ext[r0:r1, c0:c1], cos_h[:])
                nc.sync.dma_start(logits_ext[r0:r1, c0:c1], log_h[:])

            # ---- prologue emission ----
            if WARMUP_MM:
                warmup(WARMUP_MM)
            # W chain pair by pair; x tiles 0-1 interleaved after pair 1
            for pr in range(8):
                w_sumsq(pr)
                w_scale(pr)
                w_tr(pr)
                if pr == 1:
                    xb0 = x_cast(0)
                    x_tr(0, xb0)
                    xb1 = x_cast(1)
                    x_tr(1, xb1)
                if pr == 2:
                    x_sq(0)
                    x_sq(1)
                    x_norms(0)
                    normm_pair(0)
                    normm_pair(1)
                    mask(0)
                    mask(1)
                if pr == 5:
                    x_load(4, 8)
            if WARMUP2_MM:
                warmup(WARMUP2_MM)

            # ---- sweep: iteration t runs main(t,0) and main(t-CATCH,1);
            # row tile t+2 is prepped during iteration t ----
            for t in range(RT + CATCH):
                t0 = t
                t1 = t - CATCH
                if t0 < RT:
                    nt = t0 + 2
                    xb = None
                    if nt < RT:
                        if t0 % 2 == 0 and nt + 2 < RT:
                            x_load(nt + 2, min(nt + 4, RT))
                        xb = x_cast(nt)
                    ps = mq(t0, 0, 0)
                    consume(t0, 0, 0, ps)
                    ps = mq(t0, 0, 1)
                    if xb is not None:
                        x_tr(nt, xb)
                    consume(t0, 0, 1, ps)
                    store(t0, 0)
                    if nt < RT:
                        x_sq(nt)
                        if nt % 2 == 1:
                            x_norms(nt // 2)
                            normm_pair(nt - 1)
                            normm_pair(nt)
                            mask(nt - 1)
                            mask(nt)
                if 0 <= t1 < RT:
                    ps = mq(t1, 1, 0)
                    consume(t1, 1, 0, ps)
                    ps = mq(t1, 1, 1)
                    consume(t1, 1, 1, ps)
                    store(t1, 1)

    nc.finalize()
    return nc


def _in_maps(x, w, lab, mval):
    maps = []
    lab = np.asarray(lab).astype(np.int64)
    for ci in range(8):
        bi, cj = ci // NCL, ci % NCL
        b0, c0 = bi * BL, cj * CL
        ll = (lab[b0 : b0 + BL] - c0).reshape(RT, 128).T  # [128, RT]
        valid = (ll >= 0) & (ll < CL)
        labx = np.full((128, 2 * RT), -2, dtype=np.int16)
        labx[:, 0::2] = np.where(valid, ll, -1).astype(np.int16)
        maps.append(
            {
                "x": x[b0 : b0 + BL],
                "w": w[c0 : c0 + CL],
                "labx": np.ascontiguousarray(labx),
                "mvec": np.full((128, 1), mval, dtype=np.float32),
            }
        )
    return maps


def kernel(input, label, weight, s, m):
    from concourse.bass_utils import run_bass_kernel_spmd

    if "nc" not in _CACHE:
        _CACHE["nc"] = _build()
    nc = _CACHE["nc"]

    x = np.ascontiguousarray(np.asarray(input, dtype=np.float32))
    w = np.ascontiguousarray(np.asarray(weight, dtype=np.float32))
    lab = np.asarray(label)
    mval = float(np.asarray(m))

    res = run_bass_kernel_spmd(nc, _in_maps(x, w, lab, mval), core_ids=list(range(8)))

    logits = np.empty((B, C), dtype=np.float32)
    cosine = np.empty((B, C), dtype=np.float32)
    for ci in range(8):
        bi, cj = ci // NCL, ci % NCL
        b0, c0 = bi * BL, cj * CL
        logits[b0 : b0 + BL, c0 : c0 + CL] = np.asarray(
            res.results[ci]["logits"], dtype=np.float32
        )
        cosine[b0 : b0 + BL, c0 : c0 + CL] = np.asarray(
            res.results[ci]["cosine"], dtype=np.float32
        )
    return logits, cosine


# revision 12
# speedup vs baseline: 1.2322x; 1.0997x over previous
"""AMSoftmax (norm-free branch) Trainium2 kernel, 8 NeuronCores.

Reference computes, for input x [B,D], label [B], weight [C,D], scalars s,m:
    norm   = ||x||_2 per row                       [B,1]
    cosine = (x/max(norm,eps)) @ (w/max(||w||,eps)).T   [B,C]
    logits = norm * (cosine - m*onehot(label))     [B,C]
    returns (logits, cosine)

Key identity: norm * cosine == x @ w_hat.T exactly, so per output element:
    raw    = x @ w_hat.T          (PSUM, f32)
    cosine = raw * (1/norm)       (per-row scale, ACT)
    logits = raw - norm*m*onehot  (DVE sub against a sparse mask)

Sharding: 2-way over batch x 4-way over classes (8 cores, no collectives;
outputs are disjoint tiles concatenated on host). Per core: x [2048,512],
w_hat [2000,512], outputs [2048,2000] each, stored as bf16.

v5:
- x and w are shipped bf16 (host-side dtype prep, same spirit as the
  int16 label map): input DMA halves to 4.2MB and no on-chip casts.
- W normalization is folded into the PE transpose: transpose against
  diag(1/||w_c||) instead of the identity (diagonals built by GPSIMD
  scatter from a constant index map), so there is no scale-cast pass.
- Row/class sumsq are DVE ops batched over 4 (W) / 2 (X) tiles.
- Quarter-width single-bank PSUM groups, 6-deep, decouple PE from
  consumer jitter; stores ride the sync ring strictly after inputs.
- PE warm-up bursts on the identity keep the HAM clock gate open
  through the prologue.
"""

import os
import sys

sys.path.insert(0, "/opt/trn_rl_repo")

import numpy as np

B, D, C = 4096, 512, 8000
NB, NCL = 2, 4  # batch x class core grid
BL, CL = B // NB, C // NCL  # 2048, 2000 per core
RT = BL // 128  # 16 row tiles
KC = D // 128  # 4 contraction chunks
CW = 500  # matmul free-dim chunk (PSUM bank holds 512 f32)
HW_ = 2 * CW  # 1000 columns per half
NH = CL // HW_  # 2 column halves per row tile

OUT_BF16 = os.environ.get("AMS_OUT", "bf16") == "bf16"
WARMUP_MM = int(os.environ.get("AMS_WARMUP", "12"))
WARMUP2_MM = int(os.environ.get("AMS_WARMUP2", "20"))
CATCH = int(os.environ.get("AMS_CATCH", "3"))  # h=1 catch-up offset

_CACHE = {}


def _build():
    import concourse.mybir as mybir
    import concourse.tile as tile
    from concourse import bacc, library_config
    from concourse.masks import make_identity

    f32 = mybir.dt.float32
    i16 = mybir.dt.int16
    bf16 = mybir.dt.bfloat16
    odt = bf16 if OUT_BF16 else f32

    nc = bacc.Bacc()
    x_ext = nc.declare_dram_parameter("x", [BL, D], bf16, isOutput=False)
    w_ext = nc.declare_dram_parameter("w", [CL, D], bf16, isOutput=False)
    labx_ext = nc.declare_dram_parameter("labx", [128, 2 * RT], i16, isOutput=False)
    didx_ext = nc.declare_dram_parameter("didx", [128, 4], i16, isOutput=False)
    m_ext = nc.declare_dram_parameter("mvec", [128, 1], f32, isOutput=False)
    logits_ext = nc.declare_dram_parameter("logits", [BL, CL], odt, isOutput=True)
    cosine_ext = nc.declare_dram_parameter("cosine", [BL, CL], odt, isOutput=True)

    WT = (CL + 127) // 128  # 16 w row tiles (last one 80 partitions)

    with tile.TileContext(nc) as tc:
        with (
            tc.tile_pool(name="persist", bufs=1) as persist,
            tc.tile_pool(name="sq", bufs=4) as sq_pool,
            tc.tile_pool(name="psT", bufs=2, space="PSUM") as psT_pool,
            tc.tile_pool(name="psM", bufs=6, space="PSUM") as psM_pool,
            tc.tile_pool(name="outb", bufs=6) as out_pool,
            tc.tile_pool(name="mm", bufs=8) as mm_pool,
        ):
            # tiny loads ride the gpsimd SWDGE before the scatter library
            # replaces the mainline ucode
            labx_sb = persist.tile([128, 2 * RT], i16)
            didx_sb = persist.tile([128, 4], i16)
            m_sb = persist.tile([128, 1], f32)
            nc.gpsimd.dma_start(labx_sb[:], labx_ext[:])
            nc.gpsimd.dma_start(didx_sb[:], didx_ext[:])
            nc.gpsimd.dma_start(m_sb[:], m_ext[:])

            nc.gpsimd.load_library(library_config.local_scatter)
            identity = persist.tile([128, 128], bf16)
            make_identity(nc, identity)

            w_in = persist.tile([128, WT, D], bf16)
            x_in = persist.tile([128, RT, D], bf16)
            diag = persist.tile([128, WT, 128], bf16)  # diag(1/||w||) per tile
            wts = persist.tile([128, KC, CL], bf16)  # transposed normalized W
            xts = []
            for t in range(RT):
                xts.append(
                    persist.tile([128, KC, 128], bf16, tag=f"xt{t}", name=f"xt{t}")
                )

            xss = persist.tile([128, RT], f32)
            xnorm = persist.tile([128, RT], f32)
            inv_xnorm = persist.tile([128, RT], f32)
            norm_m = persist.tile([128, RT], f32)
            normm2 = persist.tile([128, 2 * RT], bf16)
            wss = persist.tile([128, WT], f32)
            inv_wnorm = persist.tile([128, WT], f32)
            inv_wbf = persist.tile([128, WT], bf16)

            nc.vector.memset(w_in[64:, WT - 1, :], 0.0)
            nc.vector.memset(wss[:], 1.0)

            # ---- x DMAs on the scalar HWDGE ring ----
            def x_load(t0, t1):
                nc.scalar.dma_start(
                    x_in[:, t0:t1, :],
                    x_ext[128 * t0 : 128 * t1, :].rearrange(
                        "(a p) d -> p a d", p=128
                    ),
                )

            x_load(0, 2)
            dumm = persist.tile([128, 1], f32)
            nc.scalar.sqrt(dumm[:], wss[:, :1])
            nc.scalar.copy(dumm[:], wss[:, :1])
            x_load(2, 4)

            # ---- W DMAs on the sync HWDGE ring (stores come later on the
            # same ring, so inputs drain first) ----
            def w_load_quad(q):  # tiles 4q..4q+3
                if q < 3:
                    nc.sync.dma_start(
                        w_in[:, 4 * q : 4 * q + 4, :],
                        w_ext[512 * q : 512 * (q + 1), :].rearrange(
                            "(a p) d -> p a d", p=128
                        ),
                    )
                else:
                    nc.sync.dma_start(
                        w_in[:, 12:15, :],
                        w_ext[1536:1920, :].rearrange("(a p) d -> p a d", p=128),
                    )
                    nc.sync.dma_start(w_in[:80, 15, :], w_ext[1920:2000, :])

            nc.sync.dma_start(
                w_in[:, 0:2, :],
                w_ext[0:256, :].rearrange("(a p) d -> p a d", p=128),
            )
            nc.sync.dma_start(
                w_in[:, 2:4, :],
                w_ext[256:512, :].rearrange("(a p) d -> p a d", p=128),
            )
            w_load_quad(1)
            w_load_quad(2)
            w_load_quad(3)

            def warmup(n):
                ps = psM_pool.tile([128, 512], f32, tag="psM")
                for _ in range(n):
                    nc.tensor.matmul(
                        ps[:, :128], identity[:], identity[:], start=True, stop=True
                    )

            # ---- W prep: batched sumsq (DVE) -> sqrt (ACT) -> recip+cast
            # (DVE) -> diag scatter (GPSIMD) -> PE transpose with diag ----
            def w_sumsq4(q):  # tiles 4q..4q+3
                a = 4 * q
                sqw = sq_pool.tile([128, 4, D], bf16, tag="sq")
                nc.vector.tensor_mul(
                    sqw[:], w_in[:, a : a + 4, :], w_in[:, a : a + 4, :]
                )
                nc.vector.reduce_sum(
                    wss[:, a : a + 4].rearrange("p (a b) -> p a b", b=1),
                    sqw[:],
                    axis=mybir.AxisListType.X,
                )
                nc.scalar.sqrt(wss[:, a : a + 4], wss[:, a : a + 4])
                cs = slice(a, a + 4)
                nc.vector.tensor_scalar_max(wss[:, cs], wss[:, cs], 1e-12)
                nc.vector.reciprocal(inv_wnorm[:, cs], wss[:, cs])
                nc.vector.tensor_copy(inv_wbf[:, cs], inv_wnorm[:, cs])
                nc.gpsimd.local_scatter(
                    diag[:, a : a + 4, :].rearrange("p a b -> p (a b)"),
                    inv_wbf[:, cs],
                    didx_sb[:],
                    channels=128,
                    num_elems=4 * 128,
                    num_idxs=4,
                )

            def w_tr_tile(a, copy_eng):  # one W row tile
                pa = min(128, CL - a * 128)
                ps = psT_pool.tile([128, KC, 128], f32, tag="psT")
                for k in range(KC):
                    # regular matmul against diag(1/||w||): psum[k, c] =
                    # sum_p w[p, k] * diag[p, c] = w[c, k] / ||w_c||
                    nc.tensor.matmul(
                        ps[:, k, :pa],
                        w_in[:pa, a, k * 128 : (k + 1) * 128],
                        diag[:pa, a, :pa],
                        start=True,
                        stop=True,
                    )
                copy_eng(
                    wts[:, :, 128 * a : 128 * a + pa], ps[:, :, :pa]
                )

            def w_tr(pr, copy_eng):  # pair of tiles 2pr, 2pr+1
                w_tr_tile(2 * pr, copy_eng)
                w_tr_tile(2 * pr + 1, copy_eng)

            # ---- X prep ----
            def x_sq2(g):  # row tiles 2g, 2g+1: sumsq + norms + normm pairs
                a = 2 * g
                sqx = sq_pool.tile([128, 2, D], bf16, tag="sq")
                nc.vector.tensor_mul(
                    sqx[:], x_in[:, a : a + 2, :], x_in[:, a : a + 2, :]
                )
                nc.vector.reduce_sum(
                    xss[:, a : a + 2].rearrange("p (a b) -> p a b", b=1),
                    sqx[:],
                    axis=mybir.AxisListType.X,
                )
                cs = slice(a, a + 2)
                nc.scalar.sqrt(xnorm[:, cs], xss[:, cs])
                nc.vector.tensor_scalar_max(xnorm[:, cs], xnorm[:, cs], 1e-12)
                nc.vector.reciprocal(inv_xnorm[:, cs], xnorm[:, cs])
                nc.vector.tensor_mul(
                    norm_m[:, cs], xnorm[:, cs], m_sb.broadcast_to([128, 2])
                )
                # bf16 [v, v] pairs for the scatter data operand, both tiles
                nc.scalar.copy(
                    normm2[:, 2 * a : 2 * a + 4].rearrange(
                        "p (a b) -> p a b", b=2
                    ),
                    norm_m[:, cs].rearrange("p (a b) -> p a b", b=1).broadcast_to(
                        [128, 2, 2]
                    ),
                )

            def x_tr(t):
                ps = psT_pool.tile([128, KC, 128], bf16, tag="psT")
                for k in range(KC):
                    nc.tensor.transpose(
                        ps[:, k, :],
                        x_in[:, t, k * 128 : (k + 1) * 128],
                        identity[:],
                    )
                nc.scalar.copy(xts[t][:], ps[:])

            masks = {}

            def mask(t):
                mmt = mm_pool.tile([128, CL], bf16, tag="mm")
                nc.gpsimd.local_scatter(
                    mmt[:],
                    normm2[:, 2 * t : 2 * t + 2],
                    labx_sb[:, 2 * t : 2 * t + 2],
                    channels=128,
                    num_elems=CL,
                    num_idxs=2,
                )
                masks[t] = mmt

            # ---- mains: quarter-width single-bank PSUM groups ----
            outs = {}

            def mq(t, h, cc):
                ps = psM_pool.tile([128, 512], f32, tag="psM")
                c0 = h * HW_ + cc * CW
                for k in range(KC):
                    nc.tensor.matmul(
                        ps[:, :CW],
                        xts[t][:, k, :],
                        wts[:, k, c0 : c0 + CW],
                        start=(k == 0),
                        stop=(k == KC - 1),
                    )
                return ps

            def consume(t, h, cc, ps):
                if (t, h) not in outs:
                    cos_h = out_pool.tile([128, HW_], odt, tag="cos")
                    log_h = out_pool.tile([128, HW_], odt, tag="log")
                    outs[(t, h)] = (cos_h, log_h)
                cos_h, log_h = outs[(t, h)]
                sl = slice(cc * CW, (cc + 1) * CW)
                c0 = h * HW_ + cc * CW
                nc.scalar.activation(
                    cos_h[:, sl],
                    ps[:, :CW],
                    mybir.ActivationFunctionType.Copy,
                    scale=inv_xnorm[:, t : t + 1],
                )
                nc.vector.tensor_sub(
                    log_h[:, sl], ps[:, :CW], masks[t][:, c0 : c0 + CW]
                )

            def store(t, h):
                cos_h, log_h = outs.pop((t, h))
                if h == NH - 1:
                    masks.pop(t)
                r0, r1 = t * 128, (t + 1) * 128
                c0, c1 = h * HW_, (h + 1) * HW_
                nc.sync.dma_start(cosine_ext[r0:r1, c0:c1], cos_h[:])
                nc.sync.dma_start(logits_ext[r0:r1, c0:c1], log_h[:])

            # ---- prologue ----
            if WARMUP_MM:
                warmup(WARMUP_MM)
            x_tr(0)
            x_tr(1)
            w_sumsq4(0)
            x_sq2(0)
            mask(0)
            mask(1)
            w_tr(0, nc.vector.tensor_copy)
            w_tr(1, nc.vector.tensor_copy)
            w_sumsq4(1)
            if WARMUP2_MM:
                warmup(WARMUP2_MM)
            w_tr(2, nc.scalar.copy)
            w_tr(3, nc.vector.tensor_copy)

            # ---- sweep: iteration t runs main(t,0) and main(t-CATCH,1);
            # row tile t+2 preps during iteration t; W quads 2-3 and pair
            # transposes 4-7 land inside iterations 0-1 ----
            for t in range(RT + CATCH):
                t0 = t
                t1 = t - CATCH
                if t0 < RT:
                    nt = t0 + 2
                    if t0 == 0:
                        w_sumsq4(2)
                    if t0 == 1:
                        w_sumsq4(3)
                    ps = mq(t0, 0, 0)
                    consume(t0, 0, 0, ps)
                    ps = mq(t0, 0, 1)
                    if nt < RT:
                        x_tr(nt)
                    consume(t0, 0, 1, ps)
                    store(t0, 0)
                    if t0 == 0:
                        w_tr(4, nc.vector.tensor_copy)
                        w_tr(5, nc.scalar.copy)
                    if t0 == 1:
                        w_tr(6, nc.vector.tensor_copy)
                        w_tr(7, nc.scalar.copy)
                    if nt < RT:
                        if t0 % 2 == 0 and nt + 2 < RT:
                            x_load(nt + 2, min(nt + 4, RT))
                        if nt % 2 == 1:
                            x_sq2(nt // 2)
                            mask(nt - 1)
                            mask(nt)
                if 0 <= t1 < RT:
                    ps = mq(t1, 1, 0)
                    consume(t1, 1, 0, ps)
                    ps = mq(t1, 1, 1)
                    consume(t1, 1, 1, ps)
                    store(t1, 1)

    nc.finalize()
    return nc


def _in_maps(x, w, lab, mval):
    import ml_dtypes

    bf = ml_dtypes.bfloat16
    maps = []
    lab = np.asarray(lab).astype(np.int64)
    didx = (np.arange(4)[None, :] * 128 + np.arange(128)[:, None]).astype(np.int16)
    xbf = np.ascontiguousarray(x.astype(bf))
    wbf = np.ascontiguousarray(w.astype(bf))
    for ci in range(8):
        bi, cj = ci // NCL, ci % NCL
        b0, c0 = bi * BL, cj * CL
        ll = (lab[b0 : b0 + BL] - c0).reshape(RT, 128).T  # [128, RT]
        valid = (ll >= 0) & (ll < CL)
        labx = np.full((128, 2 * RT), -2, dtype=np.int16)
        labx[:, 0::2] = np.where(valid, ll, -1).astype(np.int16)
        maps.append(
            {
                "x": xbf[b0 : b0 + BL],
                "w": wbf[c0 : c0 + CL],
                "labx": np.ascontiguousarray(labx),
                "didx": didx,
                "mvec": np.full((128, 1), mval, dtype=np.float32),
            }
        )
    return maps


def kernel(input, label, weight, s, m):
    from concourse.bass_utils import run_bass_kernel_spmd

    if "nc" not in _CACHE:
        _CACHE["nc"] = _build()
    nc = _CACHE["nc"]

    x = np.ascontiguousarray(np.asarray(input, dtype=np.float32))
    w = np.ascontiguousarray(np.asarray(weight, dtype=np.float32))
    lab = np.asarray(label)
    mval = float(np.asarray(m))

    res = run_bass_kernel_spmd(nc, _in_maps(x, w, lab, mval), core_ids=list(range(8)))

    logits = np.empty((B, C), dtype=np.float32)
    cosine = np.empty((B, C), dtype=np.float32)
    for ci in range(8):
        bi, cj = ci // NCL, ci % NCL
        b0, c0 = bi * BL, cj * CL
        logits[b0 : b0 + BL, c0 : c0 + CL] = np.asarray(
            res.results[ci]["logits"], dtype=np.float32
        )
        cosine[b0 : b0 + BL, c0 : c0 + CL] = np.asarray(
            res.results[ci]["cosine"], dtype=np.float32
        )
    return logits, cosine


# revision 16
# speedup vs baseline: 1.3340x; 1.0826x over previous
"""AMSoftmax (norm-free branch) Trainium2 kernel, 8 NeuronCores.

Reference computes, for input x [B,D], label [B], weight [C,D], scalars s,m:
    norm   = ||x||_2 per row                       [B,1]
    cosine = (x/max(norm,eps)) @ (w/max(||w||,eps)).T   [B,C]
    logits = norm * (cosine - m*onehot(label))     [B,C]
    returns (logits, cosine)

Key identity: norm * cosine == x @ w_hat.T exactly, so per output element:
    raw    = x @ w_hat.T          (PSUM, f32)
    cosine = raw * (1/norm)       (per-row scale, ACT)
    logits = raw - norm*m*onehot  (DVE sub against a sparse mask)

Sharding: 2-way over batch x 4-way over classes (8 cores, no collectives;
outputs are disjoint tiles concatenated on host). Per core: x [2048,512],
w_hat [2000,512], outputs [2048,2000] each, stored as bf16.

v6:
- x and w shipped bf16 (host dtype prep): input DMA is 4.2MB, no casts.
- W pairs: sumsq (alternating DVE-batched / ACT+accum), scale-cast on
  DVE, fast is_transpose into bf16 PSUM, one merged copy per pair.
- Quarter-width single-bank PSUM groups, 6-deep psM pool.
- Identity built before the GPSIMD library load; a dummy scatter right
  after the load eats the ~6us hidden IRAM fetch inside the DMA shadow.
- W pair prep for pairs 2-7 rides iterations 0-5 (CATCH=6 so h=1 starts
  after all W is ready); stores on the sync ring strictly after inputs.
- PE warm-up bursts keep the HAM clock gate open through the prologue.
"""

import os
import sys

sys.path.insert(0, "/opt/trn_rl_repo")

import numpy as np

B, D, C = 4096, 512, 8000
NB, NCL = 2, 4  # batch x class core grid
BL, CL = B // NB, C // NCL  # 2048, 2000 per core
RT = BL // 128  # 16 row tiles
KC = D // 128  # 4 contraction chunks
CW = 500  # matmul free-dim chunk (PSUM bank holds 512 f32)
HW_ = 2 * CW  # 1000 columns per half
NH = CL // HW_  # 2 column halves per row tile

OUT_BF16 = os.environ.get("AMS_OUT", "bf16") == "bf16"
WARMUP_MM = int(os.environ.get("AMS_WARMUP", "20"))
WARMUP2_MM = int(os.environ.get("AMS_WARMUP2", "12"))
CATCH = int(os.environ.get("AMS_CATCH", "5"))  # h=1 catch-up offset

_CACHE = {}


def _build():
    import concourse.mybir as mybir
    import concourse.tile as tile
    from concourse import bacc, library_config
    from concourse.masks import make_identity

    f32 = mybir.dt.float32
    i16 = mybir.dt.int16
    bf16 = mybir.dt.bfloat16
    odt = bf16 if OUT_BF16 else f32

    nc = bacc.Bacc()
    x_ext = nc.declare_dram_parameter("x", [BL, D], bf16, isOutput=False)
    w_ext = nc.declare_dram_parameter("w", [CL, D], bf16, isOutput=False)
    labx_ext = nc.declare_dram_parameter("labx", [128, 2 * RT], i16, isOutput=False)
    m_ext = nc.declare_dram_parameter("mvec", [128, 1], f32, isOutput=False)
    logits_ext = nc.declare_dram_parameter("logits", [BL, CL], odt, isOutput=True)
    cosine_ext = nc.declare_dram_parameter("cosine", [BL, CL], odt, isOutput=True)

    WT = (CL + 127) // 128  # 16 w row tiles (last one 80 partitions)

    with tile.TileContext(nc) as tc:
        with (
            tc.tile_pool(name="persist", bufs=1) as persist,
            tc.tile_pool(name="sq", bufs=4) as sq_pool,
            tc.tile_pool(name="psT", bufs=2, space="PSUM") as psT_pool,
            tc.tile_pool(name="psM", bufs=6, space="PSUM") as psM_pool,
            tc.tile_pool(name="outb", bufs=6) as out_pool,
            tc.tile_pool(name="mm", bufs=10) as mm_pool,
        ):
            # identity first (mainline gpsimd ucode), then the scatter
            # library; a throwaway scatter right after eats the hidden
            # ~6us IRAM fetch while DMAs are still in flight
            identity = persist.tile([128, 128], bf16)
            make_identity(nc, identity)
            nc.gpsimd.load_library(library_config.local_scatter)

            labx_sb = persist.tile([128, 2 * RT], i16)
            m_sb = persist.tile([128, 1], f32)

            w_in = persist.tile([128, WT, D], bf16)
            x_in = persist.tile([128, RT, D], bf16)
            w_bf = persist.tile([128, WT, D], bf16)  # normalized W
            wts = persist.tile([128, KC, CL], bf16)  # transposed normalized W
            xts = []
            for t in range(RT):
                xts.append(
                    persist.tile([128, KC, 128], bf16, tag=f"xt{t}", name=f"xt{t}")
                )

            xss = persist.tile([128, RT], f32)
            xnorm = persist.tile([128, RT], f32)
            inv_xnorm = persist.tile([128, RT], f32)
            norm_m = persist.tile([128, RT], f32)
            normm2 = persist.tile([128, 2 * RT], bf16)
            wss = persist.tile([128, WT], f32)
            inv_wnorm = persist.tile([128, WT], f32)
            scr = persist.tile([128, 16], bf16)
            scr_idx = persist.tile([128, 2], i16)

            nc.gpsimd.memset(scr_idx[:], 0)
            nc.gpsimd.local_scatter(
                scr[:],
                identity[:, 0:2],
                scr_idx[:],
                channels=128,
                num_elems=16,
                num_idxs=2,
            )

            nc.vector.memset(w_in[64:, WT - 1, :], 0.0)
            nc.vector.memset(wss[:], 1.0)

            # ---- x + tiny loads on the scalar HWDGE ring ----
            def x_load(t0, t1):
                nc.scalar.dma_start(
                    x_in[:, t0:t1, :],
                    x_ext[128 * t0 : 128 * t1, :].rearrange(
                        "(a p) d -> p a d", p=128
                    ),
                )

            x_load(0, 2)
            nc.scalar.dma_start(labx_sb[:], labx_ext[:])
            nc.scalar.dma_start(m_sb[:], m_ext[:])
            dumm = persist.tile([128, 1], f32)
            nc.scalar.sqrt(dumm[:], wss[:, :1])
            nc.scalar.copy(dumm[:], wss[:, :1])
            x_load(2, 4)

            # ---- W pair descriptors on the sync HWDGE ring ----
            def w_load_pair(pr):
                if pr < 7:
                    nc.sync.dma_start(
                        w_in[:, 2 * pr : 2 * pr + 2, :],
                        w_ext[256 * pr : 256 * (pr + 1), :].rearrange(
                            "(a p) d -> p a d", p=128
                        ),
                    )
                else:
                    nc.sync.dma_start(w_in[:, 14, :], w_ext[1792:1920, :])
                    nc.sync.dma_start(w_in[:80, 15, :], w_ext[1920:2000, :])

            for pr in range(8):
                w_load_pair(pr)

            def warmup(n):
                ps = psM_pool.tile([128, 512], f32, tag="psM")
                for _ in range(n):
                    nc.tensor.matmul(
                        ps[:, :128], identity[:], identity[:], start=True, stop=True
                    )

            # ---- W prep ----
            def w_prep(pr):
                a, b = 2 * pr, 2 * pr + 1
                pa = min(128, CL - a * 128)
                pb = min(128, CL - b * 128)
                cs = slice(a, b + 1)
                if pr % 2 == 0:  # batched sumsq on DVE
                    sqw = sq_pool.tile([128, 2, D], bf16, tag="sq")
                    nc.vector.tensor_mul(
                        sqw[:], w_in[:, cs, :], w_in[:, cs, :]
                    )
                    nc.vector.reduce_sum(
                        wss[:, cs].rearrange("p (a b) -> p a b", b=1),
                        sqw[:],
                        axis=mybir.AxisListType.X,
                    )
                else:  # per-tile Square+accum on ACT
                    for c in (a, b):
                        sqc = sq_pool.tile([128, D], bf16, tag="sq")
                        nc.scalar.activation(
                            sqc[:],
                            w_in[:, c, :],
                            mybir.ActivationFunctionType.Square,
                            accum_out=wss[:, c : c + 1],
                        )
                nc.scalar.sqrt(wss[:, cs], wss[:, cs])
                nc.vector.tensor_scalar_max(wss[:, cs], wss[:, cs], 1e-12)
                nc.vector.reciprocal(inv_wnorm[:, cs], wss[:, cs])
                nc.vector.tensor_scalar_mul(
                    w_bf[:pa, a, :], w_in[:pa, a, :], inv_wnorm[:pa, a : a + 1]
                )
                nc.vector.tensor_scalar_mul(
                    w_bf[:pb, b, :], w_in[:pb, b, :], inv_wnorm[:pb, b : b + 1]
                )

            def w_tr(pr):
                a, b = 2 * pr, 2 * pr + 1
                pa = min(128, CL - a * 128)
                pb = min(128, CL - b * 128)
                ps = psT_pool.tile([128, KC, 256], bf16, tag="psT")
                for k in range(KC):
                    nc.tensor.transpose(
                        ps[:, k, :pa],
                        w_bf[:pa, a, k * 128 : (k + 1) * 128],
                        identity[:pa, :pa],
                    )
                    nc.tensor.transpose(
                        ps[:, k, 128 : 128 + pb],
                        w_bf[:pb, b, k * 128 : (k + 1) * 128],
                        identity[:pb, :pb],
                    )
                eng = nc.vector.tensor_copy if pr % 2 == 0 else nc.scalar.copy
                if pr < 7:
                    eng(wts[:, :, 256 * pr : 256 * (pr + 1)], ps[:])
                else:
                    eng(wts[:, :, 1792:1920], ps[:, :, :128])
                    eng(wts[:, :, 1920:2000], ps[:, :, 128:208])

            # ---- X prep ----
            def x_sq2(g):  # row tiles 2g, 2g+1
                a = 2 * g
                cs = slice(a, a + 2)
                if g % 2 == 0:  # ACT per-tile Square+accum
                    for c in (a, a + 1):
                        sqc = sq_pool.tile([128, D], bf16, tag="sq")
                        nc.scalar.activation(
                            sqc[:],
                            x_in[:, c, :],
                            mybir.ActivationFunctionType.Square,
                            accum_out=xss[:, c : c + 1],
                        )
                else:  # DVE batched
                    sqx = sq_pool.tile([128, 2, D], bf16, tag="sq")
                    nc.vector.tensor_mul(
                        sqx[:], x_in[:, cs, :], x_in[:, cs, :]
                    )
                    nc.vector.reduce_sum(
                        xss[:, cs].rearrange("p (a b) -> p a b", b=1),
                        sqx[:],
                        axis=mybir.AxisListType.X,
                    )
                nc.scalar.sqrt(xnorm[:, cs], xss[:, cs])
                nc.vector.tensor_scalar_max(xnorm[:, cs], xnorm[:, cs], 1e-12)
                nc.vector.reciprocal(inv_xnorm[:, cs], xnorm[:, cs])
                nc.vector.tensor_mul(
                    norm_m[:, cs], xnorm[:, cs], m_sb.broadcast_to([128, 2])
                )
                # bf16 [v, v] pairs for the scatter data operand (GPSIMD)
                nc.gpsimd.tensor_copy(
                    normm2[:, 2 * a : 2 * a + 4].rearrange(
                        "p (a b) -> p a b", b=2
                    ),
                    norm_m[:, cs].rearrange("p (a b) -> p a b", b=1).broadcast_to(
                        [128, 2, 2]
                    ),
                )

            def x_tr(t):
                ps = psT_pool.tile([128, KC, 128], bf16, tag="psT")
                for k in range(KC):
                    nc.tensor.transpose(
                        ps[:, k, :],
                        x_in[:, t, k * 128 : (k + 1) * 128],
                        identity[:],
                    )
                if t % 2 == 0:
                    nc.scalar.copy(xts[t][:], ps[:])
                else:
                    nc.vector.tensor_copy(xts[t][:], ps[:])

            masks = {}

            def mask(t):
                mmt = mm_pool.tile([128, CL], bf16, tag="mm")
                nc.gpsimd.local_scatter(
                    mmt[:],
                    normm2[:, 2 * t : 2 * t + 2],
                    labx_sb[:, 2 * t : 2 * t + 2],
                    channels=128,
                    num_elems=CL,
                    num_idxs=2,
                )
                masks[t] = mmt

            # ---- mains: quarter-width single-bank PSUM groups ----
            outs = {}

            def mq(t, h, cc):
                ps = psM_pool.tile([128, 512], f32, tag="psM")
                c0 = h * HW_ + cc * CW
                for k in range(KC):
                    nc.tensor.matmul(
                        ps[:, :CW],
                        xts[t][:, k, :],
                        wts[:, k, c0 : c0 + CW],
                        start=(k == 0),
                        stop=(k == KC - 1),
                    )
                return ps

            def consume(t, h, cc, ps):
                if (t, h) not in outs:
                    cos_h = out_pool.tile([128, HW_], odt, tag="cos")
                    log_h = out_pool.tile([128, HW_], odt, tag="log")
                    outs[(t, h)] = (cos_h, log_h)
                cos_h, log_h = outs[(t, h)]
                sl = slice(cc * CW, (cc + 1) * CW)
                c0 = h * HW_ + cc * CW
                nc.scalar.activation(
                    cos_h[:, sl],
                    ps[:, :CW],
                    mybir.ActivationFunctionType.Copy,
                    scale=inv_xnorm[:, t : t + 1],
                )
                nc.vector.tensor_sub(
                    log_h[:, sl], ps[:, :CW], masks[t][:, c0 : c0 + CW]
                )

            def store(t, h):
                cos_h, log_h = outs.pop((t, h))
                if h == NH - 1:
                    masks.pop(t)
                r0, r1 = t * 128, (t + 1) * 128
                c0, c1 = h * HW_, (h + 1) * HW_
                nc.sync.dma_start(cosine_ext[r0:r1, c0:c1], cos_h[:])
                nc.sync.dma_start(logits_ext[r0:r1, c0:c1], log_h[:])

            # ---- prologue: W pairs 0-3 (everything h=0 touches), x tiles
            # 0-1, masks 0-1 ----
            if WARMUP_MM:
                warmup(WARMUP_MM)
            w_prep(0)
            x_tr(0)
            x_tr(1)
            w_tr(0)
            w_prep(1)
            x_sq2(0)
            mask(0)
            mask(1)
            w_tr(1)
            w_prep(2)
            w_tr(2)
            w_prep(3)
            w_tr(3)
            if WARMUP2_MM:
                warmup(WARMUP2_MM)

            # ---- sweep: iteration t runs main(t,0) and main(t-CATCH,1);
            # W pairs 4-7 prep during iterations 0-3; row tile t+2 preps
            # during iteration t ----
            for t in range(RT + CATCH):
                t0 = t
                t1 = t - CATCH
                if t0 < RT:
                    nt = t0 + 2
                    if t0 < 4:
                        w_prep(4 + t0)
                    ps = mq(t0, 0, 0)
                    consume(t0, 0, 0, ps)
                    ps = mq(t0, 0, 1)
                    if nt < RT:
                        x_tr(nt)
                    consume(t0, 0, 1, ps)
                    store(t0, 0)
                    if t0 < 4:
                        w_tr(4 + t0)
                    if nt < RT:
                        if t0 % 2 == 0 and nt + 2 < RT:
                            x_load(nt + 2, min(nt + 4, RT))
                        if nt % 2 == 1:
                            x_sq2(nt // 2)
                            mask(nt - 1)
                            mask(nt)
                if 0 <= t1 < RT:
                    ps = mq(t1, 1, 0)
                    consume(t1, 1, 0, ps)
                    ps = mq(t1, 1, 1)
                    consume(t1, 1, 1, ps)
                    store(t1, 1)

    nc.finalize()
    return nc


def _in_maps(x, w, lab, mval):
    import ml_dtypes

    bf = ml_dtypes.bfloat16
    maps = []
    lab = np.asarray(lab).astype(np.int64)
    xbf = np.ascontiguousarray(x.astype(bf))
    wbf = np.ascontiguousarray(w.astype(bf))
    for ci in range(8):
        bi, cj = ci // NCL, ci % NCL
        b0, c0 = bi * BL, cj * CL
        ll = (lab[b0 : b0 + BL] - c0).reshape(RT, 128).T  # [128, RT]
        valid = (ll >= 0) & (ll < CL)
        labx = np.full((128, 2 * RT), -2, dtype=np.int16)
        labx[:, 0::2] = np.where(valid, ll, -1).astype(np.int16)
        maps.append(
            {
                "x": xbf[b0 : b0 + BL],
                "w": wbf[c0 : c0 + CL],
                "labx": np.ascontiguousarray(labx),
                "mvec": np.full((128, 1), mval, dtype=np.float32),
            }
        )
    return maps


def kernel(input, label, weight, s, m):
    from concourse.bass_utils import run_bass_kernel_spmd

    if "nc" not in _CACHE:
        _CACHE["nc"] = _build()
    nc = _CACHE["nc"]

    x = np.ascontiguousarray(np.asarray(input, dtype=np.float32))
    w = np.ascontiguousarray(np.asarray(weight, dtype=np.float32))
    lab = np.asarray(label)
    mval = float(np.asarray(m))

    res = run_bass_kernel_spmd(nc, _in_maps(x, w, lab, mval), core_ids=list(range(8)))

    logits = np.empty((B, C), dtype=np.float32)
    cosine = np.empty((B, C), dtype=np.float32)
    for ci in range(8):
        bi, cj = ci // NCL, ci % NCL
        b0, c0 = bi * BL, cj * CL
        logits[b0 : b0 + BL, c0 : c0 + CL] = np.asarray(
            res.results[ci]["logits"], dtype=np.float32
        )
        cosine[b0 : b0 + BL, c0 : c0 + CL] = np.asarray(
            res.results[ci]["cosine"], dtype=np.float32
        )
    return logits, cosine
